# revision 7
# baseline (speedup 1.0000x reference)
"""GNN message passing via two-stage segment reduction on 8 TRN2 cores.

out[n] = sum over edges (s,d) with d==n of x[s].

Sharding: dst nodes split across 8 cores (12500 each). Host sorts each
core's edges by dst, gives node n ceil(deg/8) consecutive 8-slot
segments, pads each 128-node chunk's segment rows to NB2*128, and
gathers x rows (bf16) into slot order. Device stage 1: per superblock
(128 rows = 1024 slots), 8 matmuls against slices of a constant band
matrix W reduce each 8-slot segment to a PSUM row (no per-block one-hot
needed). Stage 2: per superblock, one small one-hot (iota compare vs
shipped node-low values) scatters the 128 segment partials into the
owning chunk's [128 nodes x 32] PSUM. bf16 operands, f32 accumulate,
bf16 output.
"""
import sys
import numpy as np

sys.path.insert(0, '/opt/trn_rl_repo')

import ml_dtypes

BF16 = np.dtype(ml_dtypes.bfloat16)

N = 100000
D = 32
NC = 8
NPC = N // NC          # 12500 dst nodes per core
CH = 128               # nodes per chunk
NCHUNK = 100           # chunks per core (98 real + 2 pad)
GC = 4                 # chunks per output staging group
NGRP = NCHUNK // GC    # 25
S = 8                  # slots per segment

_cache = {}


def _build(NB2):
    import concourse.bacc as bacc
    import concourse.tile as tile
    import concourse.mybir as mybir

    nc = bacc.Bacc("TRN2", target_bir_lowering=False, debug=False,
                   num_devices=NC)
    bf16 = mybir.dt.bfloat16
    f32 = mybir.dt.float32
    NSB = NCHUNK * NB2           # superblocks per core
    CW = NB2 * S * D             # xd cols per chunk

    xd = nc.dram_tensor("xd", (NCHUNK, 128, CW), bf16,
                        kind="ExternalInput").ap()
    meta = nc.dram_tensor("meta", (128, 240 + NSB), bf16,
                          kind="ExternalInput").ap()
    y = nc.dram_tensor("y", (NCHUNK * CH, D), bf16,
                       kind="ExternalOutput").ap()
    y_g = y.rearrange("(g cc p) f -> g p cc f", cc=GC, p=128)

    with tile.TileContext(nc) as tc:
        PIPE_C = 2                 # stage-2 lags stage-1 by 2 chunks
        XG = 2                     # chunks per input DMA
        xd2 = xd.rearrange("(cg x) p w -> cg p x w", x=XG)
        with (
            tc.tile_pool(name="const", bufs=1) as cpool,
            tc.tile_pool(name="xd", bufs=3) as xpool,
            tc.tile_pool(name="oh", bufs=PIPE_C + 2) as hpool,
            tc.tile_pool(name="pp", bufs=PIPE_C + 2) as ppool,
            tc.tile_pool(name="st", bufs=2) as spool,
            tc.tile_pool(name="ps1", bufs=2, space="PSUM") as p1pool,
            tc.tile_pool(name="ps2", bufs=2, space="PSUM") as p2pool,
        ):
            iota_t = cpool.tile([128, 128], bf16)
            nc.gpsimd.iota(iota_t[:], pattern=[[1, 128]], base=0,
                           channel_multiplier=0,
                           allow_small_or_imprecise_dtypes=True)
            meta_t = cpool.tile([128, 240 + NSB], bf16)
            nc.sync.dma_start(meta_t[:], meta[:])
            w_t = meta_t[:, 0:240]
            n2lf = cpool.tile([128, NSB], f32)
            nc.scalar.copy(n2lf[:], meta_t[:, 240:])

            cp_engs = [nc.vector.tensor_copy, nc.scalar.copy]
            st_engs = [nc.scalar.copy, nc.vector.tensor_copy]
            pts = {}
            ohs = {}
            ps2 = None
            stage = None
            for c in range(NCHUNK + PIPE_C):
                if c < NCHUNK:
                    if c % XG == 0:
                        xd_t = xpool.tile([128, XG, CW], bf16)
                        dma_eng = nc.sync if (c // XG) % 2 == 0 else nc.scalar
                        dma_eng.dma_start(xd_t[:], xd2[c // XG])
                    xv = xd_t[:, c % XG, :]
                    ps1 = p1pool.tile([128, NB2, D], f32)
                    for b2 in range(NB2):
                        for j in range(S):
                            nc.tensor.matmul(
                                ps1[:, b2, :],
                                w_t[:, 112 - 16 * j:240 - 16 * j],
                                xv[:, (b2 * S + j) * D:(b2 * S + j + 1) * D],
                                start=(j == 0), stop=(j == S - 1),
                            )
                    pt = ppool.tile([128, NB2, D], bf16)
                    cp_engs[c % 2](pt[:], ps1[:])
                    pts[c] = pt
                    oh = hpool.tile([128, NB2, 128], bf16)
                    for b2 in range(NB2):
                        sb = c * NB2 + b2
                        eng = nc.gpsimd if sb % 2 == 1 else nc.vector
                        eng.tensor_scalar(
                            oh[:, b2, :], iota_t[:], n2lf[:, sb:sb + 1],
                            None, mybir.AluOpType.is_equal,
                        )
                    ohs[c] = oh
                cd = c - PIPE_C
                if cd >= 0:
                    if cd % GC == 0:
                        stage = spool.tile([128, GC, D], bf16)
                    ps2 = p2pool.tile([128, D], f32)
                    ohd, ptd = ohs.pop(cd), pts.pop(cd)
                    for b2 in range(NB2):
                        nc.tensor.matmul(
                            ps2[:], ohd[:, b2, :], ptd[:, b2, :],
                            start=(b2 == 0), stop=(b2 == NB2 - 1),
                        )
                    st_engs[cd % 2](stage[:, cd % GC, :], ps2[:])
                    if cd % GC == GC - 1:
                        nc.sync.dma_start(y_g[cd // GC], stage[:])

    nc.compile()
    return nc


def _prep_inputs(x, edge_index):
    """Returns (in_maps, NB2)."""
    x = np.ascontiguousarray(np.asarray(x), dtype=np.float32)
    ei = np.asarray(edge_index)
    src = ei[0].astype(np.int64)
    dst = ei[1].astype(np.int64)
    xpad = np.zeros((N + 1, D), BF16)
    xpad[:N] = x.astype(BF16)

    core = dst // NPC
    per_core = []
    maxsegs = 0
    for k in range(NC):
        m = core == k
        s_k = src[m]
        d_k = dst[m] - k * NPC
        order = np.argsort(d_k, kind="stable")
        s_k, d_k = s_k[order], d_k[order]
        deg = np.bincount(d_k, minlength=NPC)
        nseg = -(-deg // S)
        segs_c = np.add.reduceat(nseg, np.arange(0, NPC, CH))
        maxsegs = max(maxsegs, int(segs_c.max()))
        per_core.append((s_k, d_k, deg, nseg))
    NB2 = max(3, -(-maxsegs // CH))
    NSB = NCHUNK * NB2
    RPC = NB2 * CH               # segment rows per chunk
    CW = NB2 * S * D

    # constant band matrix: W[p, c] = 1 iff c == p//8 + 112
    W = np.zeros((128, 240), BF16)
    W[np.arange(128), np.arange(128) // S + 112] = 1.0

    in_maps = []
    for k in range(NC):
        s_k, d_k, deg, nseg = per_core[k]
        # row start of each node (chunk-padded, node-major)
        cs = np.zeros(NPC + 1, np.int64)
        np.cumsum(nseg, out=cs[1:])
        chunk_of_n = np.arange(NPC) >> 7
        rstart = cs[:-1] + (RPC * chunk_of_n - cs[np.arange(0, NPC, CH)][chunk_of_n])
        # slot index of each (sorted) edge: 8*rstart[node] + idx_in_node
        first = np.zeros(NPC + 1, np.int64)
        np.cumsum(deg, out=first[1:])
        idx_in_node = np.arange(len(d_k)) - first[d_k]
        t = S * rstart[d_k] + idx_in_node
        xs = np.full(NSB * CH * S, N, np.int64)
        xs[t] = s_k
        # xd layout: [sb, j, p, D] -> (NCHUNK, 128, NB2, S, D)
        arr = xpad[xs].reshape(NSB, S, CH, D)          # sb, j, p, D
        arr = arr.transpose(0, 2, 1, 3)                # sb, p, j, D
        arr = arr.reshape(NCHUNK, NB2, CH, S * D)      # c, b2, p, S*D
        xdt = np.ascontiguousarray(
            arr.transpose(0, 2, 1, 3).reshape(NCHUNK, CH, CW))
        # node-low per segment row (255 = pad)
        noderow = np.full(NSB * CH, 255, np.int64)
        within = np.arange(len(d_k[:0]))  # placeholder
        cat = np.repeat(np.arange(NPC), nseg)
        within = np.arange(len(cat)) - np.repeat(cs[:-1], nseg)
        noderow[np.repeat(rstart, nseg) + within] = cat & 127
        n2l = noderow.reshape(NSB, CH).T               # [p, sb]
        metat = np.zeros((128, 240 + NSB), BF16)
        metat[:, :240] = W
        metat[:, 240:] = n2l.astype(np.float32).astype(BF16)
        in_maps.append({"xd": xdt, "meta": metat})
    return in_maps, NB2


def kernel(x, edge_index):
    from concourse import bass_utils

    in_maps, NB2 = _prep_inputs(x, edge_index)
    if NB2 not in _cache:
        _cache[NB2] = _build(NB2)
    nc = _cache[NB2]

    res = None
    for attempt in range(3):
        try:
            res = bass_utils.run_bass_kernel_spmd(nc, in_maps,
                                                  core_ids=list(range(NC)))
            break
        except Exception:
            if attempt == 2:
                raise
    out = np.empty((N, D), np.float32)
    for k in range(NC):
        out[k * NPC:(k + 1) * NPC] = (
            res.results[k]["y"][:NPC].astype(np.float32))
    return out


# revision 8
# speedup vs baseline: 1.1628x; 1.1628x over previous
"""GNN message passing via two-stage segment reduction on 8 TRN2 cores.

out[n] = sum over edges (s,d) with d==n of x[s].

Sharding: dst nodes split across 8 cores (12500 each). Host sorts each
core's edges by dst, gives node n ceil(deg/8) consecutive 8-slot
segments, pads each 128-node chunk's segment rows to NB2*128, and
gathers x rows (bf16) into slot order. Device stage 1: per superblock
(128 rows = 1024 slots), 8 matmuls against slices of a constant band
matrix W reduce each 8-slot segment to a PSUM row (no per-block one-hot
needed). Stage 2: per superblock, one small one-hot (iota compare vs
shipped node-low values) scatters the 128 segment partials into the
owning chunk's [128 nodes x 32] PSUM. bf16 operands, f32 accumulate,
bf16 output.
"""
import sys
import numpy as np

sys.path.insert(0, '/opt/trn_rl_repo')

import ml_dtypes

BF16 = np.dtype(ml_dtypes.bfloat16)

N = 100000
D = 32
NC = 8
NPC = N // NC          # 12500 dst nodes per core
CH = 128               # nodes per chunk
NCHUNK = 100           # chunks per core (98 real + 2 pad)
GC = 4                 # chunks per output staging group
NGRP = NCHUNK // GC    # 25
S = 8                  # slots per segment

_cache = {}


def _build(NB2):
    import concourse.bacc as bacc
    import concourse.tile as tile
    import concourse.mybir as mybir

    nc = bacc.Bacc("TRN2", target_bir_lowering=False, debug=False,
                   num_devices=NC)
    bf16 = mybir.dt.bfloat16
    f32 = mybir.dt.float32
    NSB = NCHUNK * NB2           # superblocks per core
    CW = NB2 * S * D             # xd cols per chunk

    xd = nc.dram_tensor("xd", (NCHUNK, 128, CW), bf16,
                        kind="ExternalInput").ap()
    meta = nc.dram_tensor("meta", (128, 240 + NSB), bf16,
                          kind="ExternalInput").ap()
    y = nc.dram_tensor("y", (NCHUNK * CH, D), bf16,
                       kind="ExternalOutput").ap()
    y_g = y.rearrange("(g cc p) f -> g p cc f", cc=GC, p=128)

    with tile.TileContext(nc) as tc:
        PIPE_C = 3                 # stage-2 lags stage-1 by 2 chunks
        XG = 2                     # chunks per input DMA
        xd2 = xd.rearrange("(cg x) p w -> cg p x w", x=XG)
        with (
            tc.tile_pool(name="const", bufs=1) as cpool,
            tc.tile_pool(name="xd", bufs=4) as xpool,
            tc.tile_pool(name="oh", bufs=PIPE_C + 2) as hpool,
            tc.tile_pool(name="pp", bufs=PIPE_C + 2) as ppool,
            tc.tile_pool(name="st", bufs=2) as spool,
            tc.tile_pool(name="ps1", bufs=3, space="PSUM") as p1pool,
            tc.tile_pool(name="ps2", bufs=3, space="PSUM") as p2pool,
        ):
            iota_t = cpool.tile([128, 128], bf16)
            nc.gpsimd.iota(iota_t[:], pattern=[[1, 128]], base=0,
                           channel_multiplier=0,
                           allow_small_or_imprecise_dtypes=True)
            meta_t = cpool.tile([128, 240 + NSB], bf16)
            nc.sync.dma_start(meta_t[:], meta[:])
            w_t = meta_t[:, 0:240]
            n2lf = cpool.tile([128, NSB], f32)
            nc.scalar.copy(n2lf[:], meta_t[:, 240:])

            cp_engs = [nc.vector.tensor_copy, nc.scalar.copy]
            st_engs = [nc.scalar.copy, nc.vector.tensor_copy]
            pts = {}
            ohs = {}
            ps2 = None
            stage = None
            for c in range(NCHUNK + PIPE_C):
                if c < NCHUNK:
                    if c % XG == 0:
                        xd_t = xpool.tile([128, XG, CW], bf16)
                        dma_eng = nc.sync if (c // XG) % 2 == 0 else nc.scalar
                        dma_eng.dma_start(xd_t[:], xd2[c // XG])
                    xv = xd_t[:, c % XG, :]
                    ps1 = p1pool.tile([128, NB2, D], f32)
                    for b2 in range(NB2):
                        for j in range(S):
                            nc.tensor.matmul(
                                ps1[:, b2, :],
                                w_t[:, 112 - 16 * j:240 - 16 * j],
                                xv[:, (b2 * S + j) * D:(b2 * S + j + 1) * D],
                                start=(j == 0), stop=(j == S - 1),
                            )
                    pt = ppool.tile([128, NB2, D], bf16)
                    cp_engs[c % 2](pt[:], ps1[:])
                    pts[c] = pt
                    oh = hpool.tile([128, NB2, 128], bf16)
                    for b2 in range(NB2):
                        sb = c * NB2 + b2
                        eng = nc.gpsimd if sb % 5 >= 3 else nc.vector
                        eng.tensor_scalar(
                            oh[:, b2, :], iota_t[:], n2lf[:, sb:sb + 1],
                            None, mybir.AluOpType.is_equal,
                        )
                    ohs[c] = oh
                cd = c - PIPE_C
                if cd >= 0:
                    if cd % GC == 0:
                        stage = spool.tile([128, GC, D], bf16)
                    ps2 = p2pool.tile([128, D], f32)
                    ohd, ptd = ohs.pop(cd), pts.pop(cd)
                    for b2 in range(NB2):
                        nc.tensor.matmul(
                            ps2[:], ohd[:, b2, :], ptd[:, b2, :],
                            start=(b2 == 0), stop=(b2 == NB2 - 1),
                        )
                    st_engs[cd % 2](stage[:, cd % GC, :], ps2[:])
                    if cd % GC == GC - 1:
                        nc.sync.dma_start(y_g[cd // GC], stage[:])

    nc.compile()
    return nc


def _prep_inputs(x, edge_index):
    """Returns (in_maps, NB2)."""
    x = np.ascontiguousarray(np.asarray(x), dtype=np.float32)
    ei = np.asarray(edge_index)
    src = ei[0].astype(np.int64)
    dst = ei[1].astype(np.int64)
    xpad = np.zeros((N + 1, D), BF16)
    xpad[:N] = x.astype(BF16)

    core = dst // NPC
    per_core = []
    maxsegs = 0
    for k in range(NC):
        m = core == k
        s_k = src[m]
        d_k = dst[m] - k * NPC
        order = np.argsort(d_k, kind="stable")
        s_k, d_k = s_k[order], d_k[order]
        deg = np.bincount(d_k, minlength=NPC)
        nseg = -(-deg // S)
        segs_c = np.add.reduceat(nseg, np.arange(0, NPC, CH))
        maxsegs = max(maxsegs, int(segs_c.max()))
        per_core.append((s_k, d_k, deg, nseg))
    NB2 = max(3, -(-maxsegs // CH))
    NSB = NCHUNK * NB2
    RPC = NB2 * CH               # segment rows per chunk
    CW = NB2 * S * D

    # constant band matrix: W[p, c] = 1 iff c == p//8 + 112
    W = np.zeros((128, 240), BF16)
    W[np.arange(128), np.arange(128) // S + 112] = 1.0

    in_maps = []
    for k in range(NC):
        s_k, d_k, deg, nseg = per_core[k]
        # row start of each node (chunk-padded, node-major)
        cs = np.zeros(NPC + 1, np.int64)
        np.cumsum(nseg, out=cs[1:])
        chunk_of_n = np.arange(NPC) >> 7
        rstart = cs[:-1] + (RPC * chunk_of_n - cs[np.arange(0, NPC, CH)][chunk_of_n])
        # slot index of each (sorted) edge: 8*rstart[node] + idx_in_node
        first = np.zeros(NPC + 1, np.int64)
        np.cumsum(deg, out=first[1:])
        idx_in_node = np.arange(len(d_k)) - first[d_k]
        t = S * rstart[d_k] + idx_in_node
        xs = np.full(NSB * CH * S, N, np.int64)
        xs[t] = s_k
        # xd layout: [sb, j, p, D] -> (NCHUNK, 128, NB2, S, D)
        arr = xpad[xs].reshape(NSB, S, CH, D)          # sb, j, p, D
        arr = arr.transpose(0, 2, 1, 3)                # sb, p, j, D
        arr = arr.reshape(NCHUNK, NB2, CH, S * D)      # c, b2, p, S*D
        xdt = np.ascontiguousarray(
            arr.transpose(0, 2, 1, 3).reshape(NCHUNK, CH, CW))
        # node-low per segment row (255 = pad)
        noderow = np.full(NSB * CH, 255, np.int64)
        within = np.arange(len(d_k[:0]))  # placeholder
        cat = np.repeat(np.arange(NPC), nseg)
        within = np.arange(len(cat)) - np.repeat(cs[:-1], nseg)
        noderow[np.repeat(rstart, nseg) + within] = cat & 127
        n2l = noderow.reshape(NSB, CH).T               # [p, sb]
        metat = np.zeros((128, 240 + NSB), BF16)
        metat[:, :240] = W
        metat[:, 240:] = n2l.astype(np.float32).astype(BF16)
        in_maps.append({"xd": xdt, "meta": metat})
    return in_maps, NB2


def kernel(x, edge_index):
    from concourse import bass_utils

    in_maps, NB2 = _prep_inputs(x, edge_index)
    if NB2 not in _cache:
        _cache[NB2] = _build(NB2)
    nc = _cache[NB2]

    res = None
    for attempt in range(3):
        try:
            res = bass_utils.run_bass_kernel_spmd(nc, in_maps,
                                                  core_ids=list(range(NC)))
            break
        except Exception:
            if attempt == 2:
                raise
    out = np.empty((N, D), np.float32)
    for k in range(NC):
        out[k * NPC:(k + 1) * NPC] = (
            res.results[k]["y"][:NPC].astype(np.float32))
    return out


# revision 9
# speedup vs baseline: 1.4411x; 1.2394x over previous
"""GNN message passing via two-stage segment reduction on 8 TRN2 cores.

out[n] = sum over edges (s,d) with d==n of x[s].

Sharding: dst nodes split across 8 cores (12500 each). Host sorts each
core's edges by dst, gives node n ceil(deg/8) consecutive 8-slot
segments, pads each 128-node chunk's segment rows to NB2*128, and
gathers x rows (bf16) into slot order. Device stage 1: per superblock
(128 rows = 1024 slots), 8 matmuls against slices of a constant band
matrix W reduce each 8-slot segment to a PSUM row (no per-block one-hot
needed). Stage 2: per superblock, one small one-hot (iota compare vs
shipped node-low values) scatters the 128 segment partials into the
owning chunk's [128 nodes x 32] PSUM. bf16 operands, f32 accumulate,
bf16 output.
"""
import sys
import numpy as np

sys.path.insert(0, '/opt/trn_rl_repo')

import ml_dtypes

BF16 = np.dtype(ml_dtypes.bfloat16)

N = 100000
D = 32
NC = 8
NPC = N // NC          # 12500 dst nodes per core
CH = 128               # nodes per chunk
NCHUNK = 100           # chunks per core (98 real + 2 pad)
GC = 4                 # chunks per output staging group
NGRP = NCHUNK // GC    # 25
S = 8                  # slots per segment

_cache = {}


def _build(NB2, TB):
    import concourse.bacc as bacc
    import concourse.tile as tile
    import concourse.mybir as mybir

    nc = bacc.Bacc("TRN2", target_bir_lowering=False, debug=False,
                   num_devices=NC)
    bf16 = mybir.dt.bfloat16
    f32 = mybir.dt.float32
    NSB = NCHUNK * NB2           # superblocks per core
    CW = TB * D                  # xd cols per chunk (TB shipped blocks)

    xd = nc.dram_tensor("xd", (NCHUNK, 128, CW), bf16,
                        kind="ExternalInput").ap()
    meta = nc.dram_tensor("meta", (128, 240 + NSB), bf16,
                          kind="ExternalInput").ap()
    y = nc.dram_tensor("y", (NCHUNK * CH, D), bf16,
                       kind="ExternalOutput").ap()
    y_g = y.rearrange("(g cc p) f -> g p cc f", cc=GC, p=128)

    with tile.TileContext(nc) as tc:
        PIPE_C = 3                 # stage-2 lags stage-1 by 2 chunks
        XG = 2                     # chunks per input DMA
        xd2 = xd.rearrange("(cg x) p w -> cg p x w", x=XG)
        with (
            tc.tile_pool(name="const", bufs=1) as cpool,
            tc.tile_pool(name="xd", bufs=4) as xpool,
            tc.tile_pool(name="oh", bufs=PIPE_C + 2) as hpool,
            tc.tile_pool(name="pp", bufs=PIPE_C + 2) as ppool,
            tc.tile_pool(name="st", bufs=2) as spool,
            tc.tile_pool(name="ps1", bufs=3, space="PSUM") as p1pool,
            tc.tile_pool(name="ps2", bufs=3, space="PSUM") as p2pool,
        ):
            iota_t = cpool.tile([128, 128], bf16)
            nc.gpsimd.iota(iota_t[:], pattern=[[1, 128]], base=0,
                           channel_multiplier=0,
                           allow_small_or_imprecise_dtypes=True)
            meta_t = cpool.tile([128, 240 + NSB], bf16)
            nc.sync.dma_start(meta_t[:], meta[:])
            w_t = meta_t[:, 0:240]
            n2lf = cpool.tile([128, NSB], f32)
            nc.scalar.copy(n2lf[:], meta_t[:, 240:])

            cp_engs = [nc.vector.tensor_copy, nc.scalar.copy]
            st_engs = [nc.scalar.copy, nc.vector.tensor_copy]
            pts = {}
            ohs = {}
            ps2 = None
            stage = None
            for c in range(NCHUNK + PIPE_C):
                if c < NCHUNK:
                    if c % XG == 0:
                        xd_t = xpool.tile([128, XG, CW], bf16)
                        dma_eng = nc.sync if (c // XG) % 2 == 0 else nc.scalar
                        dma_eng.dma_start(xd_t[:], xd2[c // XG])
                    xv = xd_t[:, c % XG, :]
                    ps1 = p1pool.tile([128, NB2, D], f32)
                    for b2 in range(NB2):
                        js = min(S, TB - b2 * S)
                        for j in range(js):
                            bk = b2 * S + j
                            nc.tensor.matmul(
                                ps1[:, b2, :],
                                w_t[:, 112 - 16 * j:240 - 16 * j],
                                xv[:, bk * D:(bk + 1) * D],
                                start=(j == 0), stop=(j == js - 1),
                            )
                    pt = ppool.tile([128, NB2, D], bf16)
                    cp_engs[c % 2](pt[:], ps1[:])
                    pts[c] = pt
                    oh = hpool.tile([128, NB2, 128], bf16)
                    for b2 in range(NB2):
                        sb = c * NB2 + b2
                        eng = nc.gpsimd if sb % 5 >= 3 else nc.vector
                        eng.tensor_scalar(
                            oh[:, b2, :], iota_t[:], n2lf[:, sb:sb + 1],
                            None, mybir.AluOpType.is_equal,
                        )
                    ohs[c] = oh
                cd = c - PIPE_C
                if cd >= 0:
                    if cd % GC == 0:
                        stage = spool.tile([128, GC, D], bf16)
                    ps2 = p2pool.tile([128, D], f32)
                    ohd, ptd = ohs.pop(cd), pts.pop(cd)
                    for b2 in range(NB2):
                        nc.tensor.matmul(
                            ps2[:], ohd[:, b2, :], ptd[:, b2, :],
                            start=(b2 == 0), stop=(b2 == NB2 - 1),
                        )
                    st_engs[cd % 2](stage[:, cd % GC, :], ps2[:])
                    if cd % GC == GC - 1:
                        nc.sync.dma_start(y_g[cd // GC], stage[:])

    nc.compile()
    return nc


def _prep_inputs(x, edge_index):
    """Returns (in_maps, NB2)."""
    x = np.ascontiguousarray(np.asarray(x), dtype=np.float32)
    ei = np.asarray(edge_index)
    src = ei[0].astype(np.int64)
    dst = ei[1].astype(np.int64)
    xpad = np.zeros((N + 1, D), BF16)
    xpad[:N] = x.astype(BF16)

    core = dst // NPC
    per_core = []
    maxsegs = 0
    for k in range(NC):
        m = core == k
        s_k = src[m]
        d_k = dst[m] - k * NPC
        order = np.argsort(d_k, kind="stable")
        s_k, d_k = s_k[order], d_k[order]
        deg = np.bincount(d_k, minlength=NPC)
        nseg = -(-deg // S)
        segs_c = np.add.reduceat(nseg, np.arange(0, NPC, CH))
        maxsegs = max(maxsegs, int(segs_c.max()))
        per_core.append((s_k, d_k, deg, nseg))
    NB2 = max(3, -(-maxsegs // CH))
    TB = max(-(-maxsegs // 16), S * (NB2 - 1) + 1)   # shipped blocks/chunk
    NSB = NCHUNK * NB2
    RPC = NB2 * CH               # segment rows per chunk
    CW = TB * D

    # constant band matrix: W[p, c] = 1 iff c == p//8 + 112
    W = np.zeros((128, 240), BF16)
    W[np.arange(128), np.arange(128) // S + 112] = 1.0

    in_maps = []
    for k in range(NC):
        s_k, d_k, deg, nseg = per_core[k]
        # row start of each node (chunk-padded, node-major)
        cs = np.zeros(NPC + 1, np.int64)
        np.cumsum(nseg, out=cs[1:])
        chunk_of_n = np.arange(NPC) >> 7
        rstart = cs[:-1] + (RPC * chunk_of_n - cs[np.arange(0, NPC, CH)][chunk_of_n])
        # slot index of each (sorted) edge: 8*rstart[node] + idx_in_node
        first = np.zeros(NPC + 1, np.int64)
        np.cumsum(deg, out=first[1:])
        idx_in_node = np.arange(len(d_k)) - first[d_k]
        t = S * rstart[d_k] + idx_in_node
        xs = np.full(NSB * CH * S, N, np.int64)
        xs[t] = s_k
        # xd layout: slot t -> (c, bk=b2*S+j, p); ship only blocks bk < TB
        arr = xpad[xs].reshape(NSB, S, CH, D)          # sb, j, p, D
        arr = arr.reshape(NCHUNK, NB2 * S, CH, D)[:, :TB]  # c, bk, p, D
        xdt = np.ascontiguousarray(
            arr.transpose(0, 2, 1, 3).reshape(NCHUNK, CH, CW))
        # node-low per segment row (255 = pad)
        noderow = np.full(NSB * CH, 255, np.int64)
        within = np.arange(len(d_k[:0]))  # placeholder
        cat = np.repeat(np.arange(NPC), nseg)
        within = np.arange(len(cat)) - np.repeat(cs[:-1], nseg)
        noderow[np.repeat(rstart, nseg) + within] = cat & 127
        n2l = noderow.reshape(NSB, CH).T               # [p, sb]
        metat = np.zeros((128, 240 + NSB), BF16)
        metat[:, :240] = W
        metat[:, 240:] = n2l.astype(np.float32).astype(BF16)
        in_maps.append({"xd": xdt, "meta": metat})
    return in_maps, (NB2, TB)


def kernel(x, edge_index):
    from concourse import bass_utils

    in_maps, key = _prep_inputs(x, edge_index)
    if key not in _cache:
        _cache[key] = _build(*key)
    nc = _cache[key]

    res = None
    for attempt in range(3):
        try:
            res = bass_utils.run_bass_kernel_spmd(nc, in_maps,
                                                  core_ids=list(range(NC)))
            break
        except Exception:
            if attempt == 2:
                raise
    out = np.empty((N, D), np.float32)
    for k in range(NC):
        out[k * NPC:(k + 1) * NPC] = (
            res.results[k]["y"][:NPC].astype(np.float32))
    return out
